# revision 3
# baseline (speedup 1.0000x reference)
"""3-layer GCN on 8 TRN2 NeuronCores (Bass/Tile).

Nodes row-sharded 12500/core, padded to 12800 (100 windows x 128). Per layer:
local linear on PE -> AllGather H table (DRAM) -> SpMM via dma_gather of H
rows (4 col-quarters to satisfy int16 indexing) + staircase one-hot scatter
matmuls accumulating into PSUM row-windows. Mean pooling is folded into a
running PE matmul against a host-scaled one-hot (1/(3*count)); AllReduce +
small dense head + softmax finish. Edge tiling is padded to a per-(window,
quarter) tile count uniform across cores so one SPMD program serves all 8.
"""

import numpy as np

N_NODES = 100000
N_GRAPHS = 64
NCORES = 8
NP = N_NODES // NCORES
NPP = 12800
NW = 100
WIN_PER_GRP = 10
NGRP = NW // WIN_PER_GRP
VFULL = NPP * NCORES
QROW = 32768
NQUART = 4
MAX_TILES_PER_CALL = 100
IN_DIM, HID, OUT_DIM = 128, 64, 10


def _host_prep(X, vals, rows, cols, batch):
    counts = np.bincount(batch, minlength=N_GRAPHS).astype(np.float64)
    inv3c = (1.0 / (3.0 * np.maximum(counts, 1.0))).astype(np.float32)

    tr_all = (cols // NP) * NPP + (cols % NP)
    q_all = tr_all // QROW
    idx_all = tr_all - q_all * QROW

    core_buckets = []
    T = np.zeros((NW, NQUART), np.int64)
    for r in range(NCORES):
        lo, hi = r * NP, (r + 1) * NP
        m = (rows >= lo) & (rows < hi)
        er = (rows[m] - lo).astype(np.int64)
        ev = vals[m].astype(np.float32)
        eq = q_all[m]
        ei = idx_all[m]
        w_all = er // 128
        ru = (er % 128).astype(np.uint8)
        order = np.lexsort((eq, w_all))
        ev, eq, ei, w_all, ru = (a[order] for a in (ev, eq, ei, w_all, ru))
        iw = np.searchsorted(w_all, np.arange(NW + 1))
        b = {}
        for w in range(NW):
            s, e = iw[w], iw[w + 1]
            jq = np.searchsorted(eq[s:e], np.arange(NQUART + 1))
            for q in range(NQUART):
                sl = slice(s + jq[q], s + jq[q + 1])
                b[(w, q)] = (ei[sl], ev[sl], ru[sl])
                T[w, q] = max(T[w, q], -(-(jq[q + 1] - jq[q]) // 128))
        core_buckets.append(b)
    for w in range(NW):
        if T[w].sum() == 0:
            T[w, 0] = 1
    win_first = {w: int(np.nonzero(T[w])[0][0]) for w in range(NW)}
    win_last = {w: int(np.nonzero(T[w])[0][-1]) for w in range(NW)}

    # uniform call schedule (same for every core)
    calls = []
    ic_off = tc_off = 0
    for g in range(NGRP):
        for q in range(NQUART):
            tiles = []
            for wl in range(WIN_PER_GRP):
                w = g * WIN_PER_GRP + wl
                for t in range(T[w, q]):
                    tiles.append((w, (q == win_first[w]) and t == 0,
                                  (q == win_last[w]) and t == T[w, q] - 1))
            p = 0
            while p < len(tiles):
                nt = min(MAX_TILES_PER_CALL, len(tiles) - p)
                calls.append(dict(g=g, q=q, ntiles=nt, tiles=tiles[p:p + nt],
                                  idx_off=ic_off, tile_off=tc_off))
                ic_off += nt * 8
                tc_off += nt
                p += nt
    total_tiles = tc_off

    per_core = []
    for r in range(NCORES):
        b = core_buckets[r]
        idx_flat = np.zeros((total_tiles, 128), np.int64)
        rows_d = np.zeros((128, total_tiles), np.uint8)
        vals_d = np.zeros((128, total_tiles), np.float32)
        tpos = 0
        for g in range(NGRP):
            for q in range(NQUART):
                for wl in range(WIN_PER_GRP):
                    w = g * WIN_PER_GRP + wl
                    n_t = T[w, q]
                    if n_t == 0:
                        continue
                    bi, bv, br = b[(w, q)]
                    n = len(bi)
                    pad = n_t * 128 - n
                    bi = np.concatenate([bi, np.zeros(pad, np.int64)])
                    bv = np.concatenate([bv, np.zeros(pad, np.float32)])
                    br = np.concatenate([br, np.zeros(pad, np.uint8)])
                    idx_flat[tpos:tpos + n_t] = bi.reshape(n_t, 128)
                    rows_d[:, tpos:tpos + n_t] = br.reshape(n_t, 128).T
                    vals_d[:, tpos:tpos + n_t] = bv.reshape(n_t, 128).T
                    tpos += n_t
        # wrapped int16 idx layout per call
        idx_cols = []
        for c in calls:
            ci = idx_flat[c["tile_off"]:c["tile_off"] + c["ntiles"]].reshape(-1)
            wrapped = ci.reshape(-1, 16).T.astype(np.int16)
            idx_cols.append(np.tile(wrapped, (8, 1)))
        idx_d = np.concatenate(idx_cols, axis=1)

        lo, hi = r * NP, (r + 1) * NP
        XT = np.zeros((IN_DIM, NPP), np.float32)
        XT[:, :NP] = X[lo:hi].T
        Po = np.zeros((NPP, N_GRAPHS), np.float32)
        Po[np.arange(NP), batch[lo:hi]] = inv3c[batch[lo:hi]]
        per_core.append(dict(XT=XT, Po=Po, idx=idx_d.astype(np.int16),
                             rows=rows_d, vals=vals_d))
    return per_core, calls, ic_off, total_tiles


def _build(calls, n_idx_cols, n_tile_cols):
    import concourse.bacc as bacc
    import concourse.bass as bass
    import concourse.mybir as mybir
    from concourse import tile
    from concourse.masks import make_identity

    nc = bacc.Bacc(dynamic_dma_scratch_size=32768, num_swdge_queues=4,
                   num_devices=NCORES)
    f32 = mybir.dt.float32
    XTd = nc.dram_tensor("XT", [IN_DIM, NPP], f32, kind="ExternalInput")
    idxd = nc.dram_tensor("idx", [128, n_idx_cols], mybir.dt.int16, kind="ExternalInput")
    rowsd = nc.dram_tensor("rowsu8", [128, n_tile_cols], mybir.dt.uint8, kind="ExternalInput")
    valsd = nc.dram_tensor("valse", [128, n_tile_cols], f32, kind="ExternalInput")
    Pod = nc.dram_tensor("Po", [NPP, N_GRAPHS], f32, kind="ExternalInput")
    W1Td = nc.dram_tensor("W1T", [IN_DIM, HID], f32, kind="ExternalInput")
    W2Td = nc.dram_tensor("W2T", [HID, HID], f32, kind="ExternalInput")
    W3Td = nc.dram_tensor("W3T", [HID, HID], f32, kind="ExternalInput")
    WoTd = nc.dram_tensor("WoT", [HID, OUT_DIM], f32, kind="ExternalInput")
    b1d = nc.dram_tensor("b1r", [128, HID], f32, kind="ExternalInput")
    b2d = nc.dram_tensor("b2r", [128, HID], f32, kind="ExternalInput")
    b3d = nc.dram_tensor("b3r", [128, HID], f32, kind="ExternalInput")
    bod = nc.dram_tensor("bor", [N_GRAPHS, OUT_DIM], f32, kind="ExternalInput")
    outd = nc.dram_tensor("out", [N_GRAPHS, OUT_DIM], f32, kind="ExternalOutput")

    Hl = nc.dram_tensor("Hl", [NPP, HID], f32)
    Hfull = nc.dram_tensor("Hfull", [VFULL, HID], f32, addr_space="Shared")
    XlT = [nc.dram_tensor(f"XlT{i}", [HID, NPP], f32) for i in range(2)]
    arin = nc.dram_tensor("arin", [HID, N_GRAPHS], f32)
    arout = nc.dram_tensor("arout", [HID, N_GRAPHS], f32, addr_space="Shared")
    rg = [list(range(NCORES))]

    with tile.TileContext(nc) as tc:
        with (
            tc.tile_pool(name="const", bufs=1) as cpool,
            tc.tile_pool(name="gbuf", bufs=3) as gpool,
            tc.tile_pool(name="sbuf", bufs=2) as spool,
            tc.tile_pool(name="ps", bufs=1, space="PSUM") as pp1,
            tc.tile_pool(name="pw", bufs=2, space="PSUM") as ppw,
        ):
            ident = cpool.tile([128, 128], f32)
            make_identity(nc, ident[:])
            iota = cpool.tile([128, 32 * 128], mybir.dt.uint8)
            nc.gpsimd.iota(iota[:], [[0, 32], [1, 128]], channel_multiplier=0,
                           allow_small_or_imprecise_dtypes=True)
            rows_t = cpool.tile([128, n_tile_cols], mybir.dt.uint8)
            nc.sync.dma_start(rows_t[:], rowsd[:])
            vals_t = cpool.tile([128, n_tile_cols], f32)
            nc.sync.dma_start(vals_t[:], valsd[:])
            wts, breps = [], []
            for j, (d, k) in enumerate(((W1Td, IN_DIM), (W2Td, HID), (W3Td, HID))):
                t = cpool.tile([k, HID], f32, tag=f"w{j}")
                nc.sync.dma_start(t[:], d[:])
                wts.append(t)
            for j, d in enumerate((b1d, b2d, b3d)):
                t = cpool.tile([128, HID], f32, tag=f"b{j}")
                nc.sync.dma_start(t[:], d[:])
                breps.append(t)
            wot = cpool.tile([HID, OUT_DIM], f32)
            nc.sync.dma_start(wot[:], WoTd[:])
            bot = cpool.tile([N_GRAPHS, OUT_DIM], f32)
            nc.sync.dma_start(bot[:], bod[:])

            pooled_ps = pp1.tile([HID, N_GRAPHS], f32, tag="pool")

            for layer in range(3):
                K = IN_DIM if layer == 0 else HID
                src = XTd if layer == 0 else XlT[layer - 1]
                for i in range(NW):
                    lx = spool.tile([K, 128], f32, tag="lx")
                    nc.sync.dma_start(lx[:], src[:, i * 128:(i + 1) * 128])
                    ph = pp1.tile([128, HID], f32, tag="ph")
                    nc.tensor.matmul(ph[:], lhsT=lx[:], rhs=wts[layer][:],
                                     start=True, stop=True)
                    hsb = spool.tile([128, HID], f32, tag="hsb")
                    nc.vector.tensor_tensor(out=hsb[:], in0=ph[:],
                                            in1=breps[layer][:],
                                            op=mybir.AluOpType.add)
                    nc.sync.dma_start(Hl[i * 128:(i + 1) * 128, :], hsb[:])
                nc.gpsimd.collective_compute(
                    "AllGather", mybir.AluOpType.bypass, rg,
                    ins=[Hl[:]], outs=[Hfull[:]])
                for g in range(NGRP):
                    pw = ppw.tile([128, WIN_PER_GRP * HID], f32, tag="pw")
                    for c in calls:
                        if c["g"] != g:
                            continue
                        q = c["q"]; nt = c["ntiles"]; NIc = nt * 128
                        it = spool.tile([128, MAX_TILES_PER_CALL * 8],
                                        mybir.dt.int16, tag="it")
                        nc.sync.dma_start(
                            it[:, :nt * 8],
                            idxd[:, c["idx_off"]:c["idx_off"] + nt * 8])
                        gt = gpool.tile([128, MAX_TILES_PER_CALL, HID], f32, tag="g")
                        qrows = min(QROW, VFULL - q * QROW)
                        nc.gpsimd.dma_gather(
                            gt[:, :nt, :], Hfull[q * QROW:q * QROW + qrows, :],
                            it[:, :nt * 8], NIc, NIc, HID,
                            single_packet=False, queue_num=q)
                        vb = vals_t[:, c["tile_off"]:c["tile_off"] + nt]
                        vb = bass.AP(vb.tensor, vb.offset, list(vb.ap) + [[0, HID]])
                        nc.vector.tensor_tensor(out=gt[:, :nt, :], in0=gt[:, :nt, :],
                                                in1=vb, op=mybir.AluOpType.mult)
                        for s0 in range(0, nt, 32):
                            ns = min(32, nt - s0)
                            S = spool.tile([128, 32 * 128], f32, tag="S")
                            rb = rows_t[:, c["tile_off"] + s0:c["tile_off"] + s0 + ns]
                            rb = bass.AP(rb.tensor, rb.offset,
                                         list(rb.ap) + [[0, 128]])
                            nc.vector.tensor_tensor(
                                out=S[:, :ns * 128].rearrange(
                                    "p (t r) -> p t r", r=128),
                                in0=rb,
                                in1=iota[:, :ns * 128].rearrange(
                                    "p (t r) -> p t r", r=128),
                                op=mybir.AluOpType.is_equal)
                            for t in range(ns):
                                w, first, last = c["tiles"][s0 + t]
                                wl = w % WIN_PER_GRP
                                nc.tensor.matmul(
                                    pw[:, wl * HID:(wl + 1) * HID],
                                    lhsT=S[:, t * 128:(t + 1) * 128],
                                    rhs=gt[:, s0 + t, :],
                                    start=first, stop=last)
                    for wl in range(WIN_PER_GRP):
                        w = g * WIN_PER_GRP + wl
                        xsb = spool.tile([128, HID], f32, tag="xsb")
                        nc.scalar.activation(xsb[:], pw[:, wl * HID:(wl + 1) * HID],
                                             mybir.ActivationFunctionType.Relu)
                        po = spool.tile([128, N_GRAPHS], f32, tag="po")
                        nc.sync.dma_start(po[:], Pod[w * 128:(w + 1) * 128, :])
                        nc.tensor.matmul(pooled_ps[:], lhsT=xsb[:], rhs=po[:],
                                         start=(layer == 0 and w == 0),
                                         stop=(layer == 2 and w == NW - 1))
                        if layer < 2:
                            tp = pp1.tile([HID, 128], f32, tag="tp")
                            nc.tensor.transpose(tp[:], xsb[:], ident[:])
                            xts = spool.tile([HID, 128], f32, tag="xts")
                            nc.vector.tensor_copy(xts[:], tp[:])
                            nc.sync.dma_start(
                                XlT[layer][:, w * 128:(w + 1) * 128], xts[:])

            pg = spool.tile([HID, N_GRAPHS], f32, tag="pg")
            nc.vector.tensor_copy(pg[:], pooled_ps[:])
            nc.sync.dma_start(arin[:], pg[:])
            nc.gpsimd.collective_compute("AllReduce", mybir.AluOpType.add, rg,
                                         ins=[arin[:]], outs=[arout[:]])
            pga = spool.tile([HID, N_GRAPHS], f32, tag="pga")
            nc.sync.dma_start(pga[:], arout[:])
            lg = pp1.tile([N_GRAPHS, OUT_DIM], f32, tag="lg")
            nc.tensor.matmul(lg[:], lhsT=pga[:], rhs=wot[:], start=True, stop=True)
            ls = spool.tile([N_GRAPHS, OUT_DIM], f32, tag="ls")
            nc.vector.tensor_tensor(out=ls[:], in0=lg[:], in1=bot[:],
                                    op=mybir.AluOpType.add)
            mx = spool.tile([N_GRAPHS, 1], f32, tag="mx")
            nc.vector.tensor_reduce(mx[:], ls[:], axis=mybir.AxisListType.X,
                                    op=mybir.AluOpType.max)
            nc.vector.tensor_tensor(out=ls[:], in0=ls[:],
                                    in1=mx[:].to_broadcast([N_GRAPHS, OUT_DIM]),
                                    op=mybir.AluOpType.subtract)
            nc.scalar.activation(ls[:], ls[:], mybir.ActivationFunctionType.Exp)
            sm = spool.tile([N_GRAPHS, 1], f32, tag="sm")
            nc.vector.tensor_reduce(sm[:], ls[:], axis=mybir.AxisListType.X,
                                    op=mybir.AluOpType.add)
            nc.vector.reciprocal(sm[:], sm[:])
            nc.vector.tensor_tensor(out=ls[:], in0=ls[:],
                                    in1=sm[:].to_broadcast([N_GRAPHS, OUT_DIM]),
                                    op=mybir.AluOpType.mult)
            nc.sync.dma_start(outd[:], ls[:])
    nc.finalize()
    return nc


def kernel(X, vals, W1, b1, W2, b2, W3, b3, Wout, bout, rows, cols, batch):
    from concourse.bass_utils import run_bass_kernel_spmd

    X = np.asarray(X, np.float32)
    per_core, calls, n_idx_cols, n_tile_cols = _host_prep(
        X, np.asarray(vals), np.asarray(rows), np.asarray(cols),
        np.asarray(batch))
    nc = _build(calls, n_idx_cols * 16 // 16, n_tile_cols)

    common = dict(
        W1T=np.ascontiguousarray(np.asarray(W1, np.float32).T),
        W2T=np.ascontiguousarray(np.asarray(W2, np.float32).T),
        W3T=np.ascontiguousarray(np.asarray(W3, np.float32).T),
        WoT=np.ascontiguousarray(np.asarray(Wout, np.float32).T),
        b1r=np.tile(np.asarray(b1, np.float32), (128, 1)),
        b2r=np.tile(np.asarray(b2, np.float32), (128, 1)),
        b3r=np.tile(np.asarray(b3, np.float32), (128, 1)),
        bor=np.tile(np.asarray(bout, np.float32), (N_GRAPHS, 1)),
    )
    in_maps = []
    for p in per_core:
        m = dict(common)
        m.update(XT=p["XT"], idx=p["idx"], rowsu8=p["rows"], valse=p["vals"],
                 Po=p["Po"])
        in_maps.append(m)
    res = run_bass_kernel_spmd(nc, in_maps, core_ids=list(range(NCORES)))
    return np.asarray(res.results[0]["out"], np.float32)
